# revision 35
# baseline (speedup 1.0000x reference)
"""Trainium2 Bass kernel for the soft-logic cellular-automaton nn.Module.

Reference semantics (B=16, M=4096, N=8192, K=6, P=64, L=8, STEP=2):
    tw = sigmoid(toggle_gates)                      # (L, P, N)
    state = zeros(B, N); state[:, ::2] = x
    for l in range(L):
        win[b,n,i] = state[b, (n+i-2) mod N]        # i in 0..5
        w[b,n,p]   = prod_i (bit_i(p) ? win_i : 1-win_i)
        state[b,n] = clip(sum_p w[b,n,p]*tw[l,p,n], 0, 1)
    return state[:, ::2]

Sharding: grid dim N split across 8 cores (1024 owned columns each).
Each core computes a halo-grown region (2 left / 3 right per layer -> 16/24
total) so NO inter-core communication is needed during the 8 layers.

On-core layout ("F-major"): 128 partitions = (b=16) x (chunk c=8); each
partition holds a contiguous 168-column n-window (128 owned + 40 halo) on
the free dim, so the 6 convolution taps are plain AP column offsets.
State+complement live in one paired tile SC[128, 2, W0] (row0 = 1-state,
row1 = state) so a tap selected by a combo bit is a stride-able AP dim.

Per layer the 64-term contraction  sum_p wA[pa]*wB16[pb]*tw[p,n]  (2+4 bit
split: pa = taps 0-1, pb = taps 2-5) runs on the vector engine in 8 big
multi-dim ops using 0-stride broadcast views: 4 pair-tree muls build
wA[4]/wB16[16], two (mul vs tw + segmented tensor_reduce over pb) halves
give g[pa], then one final mul+reduce over pa.  Layer 0 exploits the
stride-2 embedding (odd slots exactly 0/1): only 8 combos per output
parity survive, computed from stride-2 views against a compact 16-combo
toggle tensor.  clip is skipped: tw in (0.5, 0.732) and sum_p w = 1
exactly, so outputs stay inside (0,1).

toggle weights are streamed from DRAM per layer by ONE broadcast DMA
(0-stride b-replication onto all 128 partitions, all 16 SDMA engines),
prefetched TWO layers ahead (bufs=3), and sigmoid-ed in place in two
halves on the scalar engine so consumer muls gate on half the sigmoid.
"""

import os
import sys
from contextlib import ExitStack

import numpy as np

for _p in ("/opt/trn_rl_repo", "/root/.axon_site/_ro/trn_rl_repo"):
    if os.path.isdir(_p) and _p not in sys.path:
        sys.path.insert(0, _p)

import concourse.bass as bass  # noqa: E402
import concourse.tile as tile  # noqa: E402
from concourse import bacc, mybir  # noqa: E402
from concourse.bass_utils import run_bass_kernel_spmd  # noqa: E402

B, M, N, KK, P, L = 16, 4096, 8192, 6, 64, 8
NCORES = 8
NOWN = N // NCORES          # 1024 owned grid columns per core
NCHUNK = 8                  # chunks (partitions per batch row)
CHUNK = NOWN // NCHUNK      # 128 owned columns per partition
GROW_L, GROW_R = 2 * L, 3 * L   # 16, 24
W0 = CHUNK + GROW_L + GROW_R    # 168 column window at layer 0
XW = W0 // 2                    # 84 even columns carrying x
F32 = mybir.dt.float32

# variant knobs: (batched tree/mul via 0-stride views, #pa-groups on gpsimd,
#                 sparse layer-0 using the zero-interleave structure)
DEFAULT_VARIANT = dict(batched=True, gp_pa=0, sparse_l0=True, dve_comp=True,
                       half_l7=True)


def _build_program(reps=1, batched=False, gp_pa=0, sparse_l0=False, probe="",
                   dve_comp=False, sig2=False, addtree=False, half_l7=False):
    nc = bacc.Bacc("TRN2", target_bir_lowering=False, debug=False)
    xs = nc.dram_tensor("xs", [128, XW], F32, kind="ExternalInput").ap()
    tg = nc.dram_tensor("tg", [L, NCHUNK, W0, P], F32, kind="ExternalInput").ap()
    # layer-0 compact toggles: [parity, chunk, e, combo] (16 surviving combos)
    tg0 = nc.dram_tensor("tg0", [NCHUNK, 2, XW, 8], F32, kind="ExternalInput").ap()
    # layer-7 toggles for even output columns only: [chunk, e, p]
    tg7 = nc.dram_tensor("tg7", [NCHUNK, CHUNK // 2, P], F32, kind="ExternalInput").ap()
    out = nc.dram_tensor("out", [128, CHUNK // 2], F32, kind="ExternalOutput").ap()

    mult = mybir.AluOpType.mult
    add = mybir.AluOpType.add
    AX = mybir.AxisListType.X
    AF = mybir.ActivationFunctionType

    with tile.TileContext(nc) as tc, ExitStack() as ctx:
        pool = ctx.enter_context(tc.tile_pool(name="work", bufs=1))
        twpool = ctx.enter_context(tc.tile_pool(name="tw", bufs=3))

        # paired state tiles: row0 = comp (1-state), row1 = state
        SC = [pool.tile([128, 2, W0], F32, name="scA", tag="scA"),
              pool.tile([128, 2, W0], F32, name="scB", tag="scB")]
        tmp4 = pool.tile([128, 4, W0], F32, name="tmp4", tag="tmp4")
        tmp23 = pool.tile([128, 4, W0], F32, name="tmp23", tag="tmp23")
        tmp45 = pool.tile([128, 4, W0], F32, name="tmp45", tag="tmp45")
        wa = pool.tile([128, W0, 8], F32, name="wa", tag="wa")      # (j, pa)
        wb = pool.tile([128, W0, 8], F32, name="wb", tag="wb")      # (j, pb)
        wb16 = pool.tile([128, W0, 16], F32, name="wb16", tag="wb16")  # (j, pb4)
        p64 = pool.tile([128, 2, W0, 16], F32, name="p64", tag="p64")  # (pa, j, pb)
        gf = pool.tile([128, W0, 8], F32, name="gf", tag="gf")      # (j, pa)
        fin = pool.tile([128, W0, 8], F32, name="fin", tag="fin")   # (j, pa)

        # state init: zeros with x at even columns (SWDGE queue: stays off the
        # critical HWDGE queue carrying the first big toggle fetch)
        nc.vector.memset(SC[0][:], 0.0)
        nc.gpsimd.dma_start(out=SC[0][:, 1, 0:W0:2], in_=xs[:, :])

        tw_tiles = {}

        def pruned(gl):
            return half_l7 and gl % L == L - 1

        def fetch_tw(gl):
            t = twpool.tile([128, W0, P], F32, name="twt", tag="tw")
            if probe != "nodma" or gl <= 1:
                if pruned(gl):
                    # host pre-packed even-column toggles, contiguous on chip
                    nc.sync.dma_start(out=t[:, 0:CHUNK // 2, :],
                                      in_=tg7.partition_broadcast(16))
                else:
                    ll = gl % L
                    lo, ro = 2 * ll + 2, W0 - 3 * ll - 3
                    # single DMA, b-replication via 0-stride src dim; fetch
                    # only the consumed column window
                    nc.sync.dma_start(
                        out=t[:, lo:ro, :],
                        in_=tg[gl % L][:, lo:ro, :].partition_broadcast(16))
            tw_tiles[gl] = t

        def sigmoid_tw(gl, half):
            if probe == "nosig":
                return
            if pruned(gl):
                lo, ro = 0, CHUNK // 2
            else:
                ll = gl % L
                lo, ro = 2 * ll + 2, W0 - 3 * ll - 3
            t = tw_tiles[gl]
            sl = slice(32 * half, 32 * half + 32)
            nc.scalar.activation(t[:, lo:ro, sl], t[:, lo:ro, sl], AF.Sigmoid)

        def needs_tw(gl):
            return gl < L * reps and not (sparse_l0 and gl % L == 0)

        if sparse_l0:
            tw0 = pool.tile([128, 2, XW, 8], F32, name="tw0", tag="tw0")
            nc.gpsimd.dma_start(
                out=tw0[:, :, :, :],
                in_=tg0.partition_broadcast(16))
            nc.scalar.activation(tw0[:, :, :, :], tw0[:, :, :, :], AF.Sigmoid)
        else:
            fetch_tw(0)
            sigmoid_tw(0, 0)
            sigmoid_tw(0, 1)
        if needs_tw(1):
            fetch_tw(1)
            if sig2:
                sigmoid_tw(1, 0)
                sigmoid_tw(1, 1)

        for gl in range(L * reps):
            l = gl % L
            lin, rin = 2 * l, W0 - 3 * l
            lo, ro = lin + 2, rin - 3
            wo = ro - lo
            sin, sout = SC[gl % 2], SC[(gl + 1) % 2]

            # prefetch toggle gates TWO layers ahead (bufs=3) so next layer's
            # sigmoid never waits on its DMA
            if needs_tw(gl + 2):
                fetch_tw(gl + 2)

            # comp = 1 - state on the input window. On DVE (tensor_scalar,
            # single-src 2x path) the fin-reduce -> comp -> tree chain stays
            # on one engine: no cross-engine semaphore bubble per layer, and
            # ACT's FIFO holds only sigmoids.
            if dve_comp:
                nc.vector.tensor_scalar(sin[:, 0, lin:rin], sin[:, 1, lin:rin],
                                        -1.0, 1.0, mult, add)
            else:
                nc.scalar.activation(sin[:, 0, lin:rin], sin[:, 1, lin:rin],
                                     AF.Identity, bias=1.0, scale=-1.0)

            # sigmoid queues on ACT in two halves so consumer big-muls gate
            # on half the work; with sig2 it runs a full extra layer early
            sgl = gl + 2 if sig2 else gl + 1
            if needs_tw(sgl):
                sigmoid_tw(sgl, 0)
                sigmoid_tw(sgl, 1)

            if sparse_l0 and l == 0:
                # Layer 0: odd grid slots are exactly 0 (state) / 1 (comp), so
                # only 8 of 64 combos survive per output parity; taps collapse
                # to stride-2 views of the x-carrying even slots.
                # even outputs j=2e, e in [1,82]: taps at even slots e-1,e,e+1
                VE = [sin[:, :, 2 * d: 2 * d + 164: 2] for d in (0, 1, 2)]
                t4e = tmp4.rearrange("p (a b) j -> p a b j", a=2)[:, :, :, 0:82]
                nc.vector.tensor_tensor(
                    t4e,
                    VE[0].unsqueeze(2).broadcast_to((128, 2, 2, 82)),
                    VE[1].unsqueeze(1).broadcast_to((128, 2, 2, 82)), mult)
                wav = wa[:, 0:82, 0:8].rearrange("p j (q b) -> p q b j", q=4)
                nc.vector.tensor_tensor(
                    wav,
                    tmp4[:, :, 0:82].unsqueeze(2).broadcast_to((128, 4, 2, 82)),
                    VE[2].unsqueeze(1).broadcast_to((128, 4, 2, 82)),
                    mult)
                nc.vector.tensor_tensor(p64[:, 0, 0:82, 0:8], wa[:, 0:82, :],
                                        tw0[:, 0, 1:83, :], mult)
                nc.vector.tensor_reduce(sout[:, 1, 2:165:2], p64[:, 0, 0:82, 0:8],
                                        axis=AX, op=add)
                # odd outputs j=2e+1, e in [1,81]: taps at even slots e,e+1,e+2
                VO = [sin[:, :, 2 * d + 2: 2 * d + 164: 2] for d in (0, 1, 2)]
                t4o = tmp4.rearrange("p (a b) j -> p a b j", a=2)[:, :, :, 0:81]
                nc.vector.tensor_tensor(
                    t4o,
                    VO[0][:, :, 0:81].unsqueeze(2).broadcast_to((128, 2, 2, 81)),
                    VO[1][:, :, 0:81].unsqueeze(1).broadcast_to((128, 2, 2, 81)), mult)
                wbv = wb[:, 0:81, 0:8].rearrange("p j (q b) -> p q b j", q=4)
                nc.vector.tensor_tensor(
                    wbv,
                    tmp4[:, :, 0:81].unsqueeze(2).broadcast_to((128, 4, 2, 81)),
                    VO[2][:, :, 0:81].unsqueeze(1).broadcast_to((128, 4, 2, 81)),
                    mult)
                nc.vector.tensor_tensor(p64[:, 1, 0:81, 0:8], wb[:, 0:81, :],
                                        tw0[:, 1, 1:82, :], mult)
                nc.vector.tensor_reduce(sout[:, 1, 3:164:2], p64[:, 1, 0:81, 0:8],
                                        axis=AX, op=add)
                continue

            twl = tw_tiles[gl]

            def V(i, bit):
                # [128, wo] view of tap i (bit=1: state, 0: comp)
                return sin[:, bit, lin + i: lin + i + wo]

            # last layer: only even grid columns are ever read out, so
            # compute just those (all views become stride-2; volume halves)
            js = 2 if (half_l7 and l == L - 1) else 1
            wos = wo // js

            def VP(i):
                # [128, 2, wos] view of tap i, dim1 selects comp/state
                return sin[:, :, lin + i: lin + i + wo: js]

            if batched:
                # --- 2+4 bit split: wA = taps 0,1 (4 combos, = tmp4),
                #     wB16 = taps 2..5 (16 combos) built from two pair trees ---
                t4v = tmp4.rearrange("p (a b) j -> p a b j", a=2)[:, :, :, 0:wos]
                nc.vector.tensor_tensor(
                    t4v,
                    VP(0).unsqueeze(2).broadcast_to((128, 2, 2, wos)),
                    VP(1).unsqueeze(1).broadcast_to((128, 2, 2, wos)), mult)
                t23v = tmp23.rearrange("p (a b) j -> p a b j", a=2)[:, :, :, 0:wos]
                nc.vector.tensor_tensor(
                    t23v,
                    VP(2).unsqueeze(2).broadcast_to((128, 2, 2, wos)),
                    VP(3).unsqueeze(1).broadcast_to((128, 2, 2, wos)), mult)
                t45v = tmp45.rearrange("p (a b) j -> p a b j", a=2)[:, :, :, 0:wos]
                nc.vector.tensor_tensor(
                    t45v,
                    VP(4).unsqueeze(2).broadcast_to((128, 2, 2, wos)),
                    VP(5).unsqueeze(1).broadcast_to((128, 2, 2, wos)), mult)
                wb16v = wb16[:, 0:wos, :].rearrange("p j (q b) -> p q b j", q=4)
                nc.vector.tensor_tensor(
                    wb16v,
                    tmp23[:, :, 0:wos].unsqueeze(2).broadcast_to((128, 4, 4, wos)),
                    tmp45[:, :, 0:wos].unsqueeze(1).broadcast_to((128, 4, 4, wos)),
                    mult)

                # --- products vs tw + segmented reduce, in two halves gated on
                #     the two sigmoid halves ---
                for h in range(2):
                    nc.vector.tensor_tensor(
                        p64[:, :, 0:wos, :],
                        wb16[:, 0:wos, :].unsqueeze(1).broadcast_to(
                            (128, 2, wos, 16)),
                        (twl[:, 0:wos, :] if js == 2 else twl[:, lo:ro, :])
                        [:, :, 32 * h:32 * h + 32].rearrange(
                            "p j (a b) -> p a j b", a=2), mult)
                    gv = gf[:, 0:wos, 2 * h:2 * h + 2].rearrange("p j a -> p a j")
                    if addtree:
                        # pairwise in-place TT adds instead of tensor_reduce
                        for wdt in (8, 4, 2):
                            nc.vector.tensor_tensor(
                                p64[:, :, 0:wos, 0:wdt], p64[:, :, 0:wos, 0:wdt],
                                p64[:, :, 0:wos, wdt:2 * wdt], add)
                        nc.vector.tensor_tensor(
                            gv, p64[:, :, 0:wos, 0], p64[:, :, 0:wos, 1], add)
                    else:
                        nc.vector.tensor_reduce(
                            gv, p64[:, :, 0:wos, :], axis=AX, op=add)

                # --- out = sum_{pa in 4} wA[pa] * g[pa] ---
                nc.vector.tensor_tensor(
                    fin[:, 0:wos, 0:4].rearrange("p j a -> p a j"),
                    tmp4[:, :, 0:wos],
                    gf[:, 0:wos, 0:4].rearrange("p j a -> p a j"), mult)
                nc.vector.tensor_reduce(sout[:, 1, lo:ro:js], fin[:, 0:wos, 0:4],
                                        axis=AX, op=add)
                continue
            else:
                for q in range(4):
                    nc.vector.tensor_tensor(tmp4[:, q, 0:wo], V(0, q >> 1), V(1, q & 1), mult)
                for pa in range(8):
                    nc.vector.tensor_tensor(wa[:, 0:wo, pa], tmp4[:, pa >> 1, 0:wo], V(2, pa & 1), mult)
                for q in range(4):
                    nc.vector.tensor_tensor(tmp4[:, q, 0:wo], V(3, q >> 1), V(4, q & 1), mult)
                for pb in range(8):
                    nc.vector.tensor_tensor(wb[:, 0:wo, pb], tmp4[:, pb >> 1, 0:wo], V(5, pb & 1), mult)

            # --- unbatched fallback: per-pa products vs tw, gpsimd takes the
            #     LAST gp_pa groups (p64 viewed as 8 groups of 8) ---
            def pv(pa):
                return p64[:, (pa >> 1) & 1, 0:wo, 8 * (pa & 1):8 * (pa & 1) + 8]

            for pa in range(8):
                eng = nc.gpsimd if pa >= 8 - gp_pa else nc.vector
                eng.tensor_tensor(
                    pv(pa), wb[:, 0:wo, :],
                    twl[:, lo:ro, pa * 8:(pa + 1) * 8], mult)
                nc.vector.tensor_reduce(
                    gf[:, 0:wo, pa], pv(pa), axis=AX, op=add)

            # --- out = sum_pa wA[pa] * g[pa] ---
            nc.vector.tensor_tensor(fin[:, 0:wo, :], wa[:, 0:wo, :], gf[:, 0:wo, :], mult)
            nc.vector.tensor_reduce(sout[:, 1, lo:ro], fin[:, 0:wo, 0:8], axis=AX, op=add)

        # owned even columns -> output
        nc.sync.dma_start(out=out, in_=SC[(L * reps) % 2][:, 1, GROW_L:GROW_L + CHUNK:2])

    nc.compile()
    return nc


_prog_cache = {}


def _get_program(reps=1, **variant):
    v = dict(DEFAULT_VARIANT)
    v.update(variant)
    key = (reps, tuple(sorted(v.items())))
    if key not in _prog_cache:
        _prog_cache[key] = _build_program(reps, **v)
    return _prog_cache[key]


def _shard_inputs(x, toggle_gates):
    x = np.ascontiguousarray(x, dtype=np.float32)
    tg = np.ascontiguousarray(toggle_gates, dtype=np.float32)
    in_maps = []
    c = np.arange(NCHUNK)
    j = np.arange(W0)
    # layer-0 surviving combos (even outputs: bits 1,3,5 = 0; odd: bits 0,2,4 = 0)
    p_even = np.array([32 * (q >> 2) + 8 * ((q >> 1) & 1) + 2 * (q & 1)
                       for q in range(8)])
    p_odd = np.array([16 * (q >> 2) + 4 * ((q >> 1) & 1) + (q & 1)
                      for q in range(8)])
    for k in range(NCORES):
        n0 = k * NOWN
        nglob = (n0 + CHUNK * c[:, None] - GROW_L + j[None, :]) % N  # [8, 168]
        m_idx = nglob[:, 0::2] // 2                                   # [8, 84]
        xs = x[:, m_idx].reshape(B * NCHUNK, XW)                      # [128, 84]
        tgk = tg[:, :, nglob]                                         # [L, P, 8, 168]
        tg0 = np.stack([tgk[0, p_even][:, :, 0::2],                   # [8q, 8c, 84]
                        tgk[0, p_odd][:, :, 1::2]])                   # [2, 8q, 8c, 84]
        tg0 = np.ascontiguousarray(tg0.transpose(2, 0, 3, 1))         # [8c, 2, 84, 8q]
        tg7 = np.ascontiguousarray(
            tgk[L - 1][:, :, GROW_L:GROW_L + CHUNK:2].transpose(1, 2, 0))  # [8c, 64e, P]
        tgk = np.ascontiguousarray(tgk.transpose(0, 2, 3, 1))         # [L, 8, 168, P]
        in_maps.append({"xs": np.ascontiguousarray(xs), "tg": tgk, "tg0": tg0,
                        "tg7": tg7})
    return in_maps


def _run(x, toggle_gates, trace=False, reps=1, **kw):
    nc = _get_program(reps, **kw)
    in_maps = _shard_inputs(x, toggle_gates)
    res = run_bass_kernel_spmd(nc, in_maps, list(range(NCORES)), trace=trace)
    y = np.empty((B, M), dtype=np.float32)
    for k in range(NCORES):
        o = np.asarray(res.results[k]["out"]).reshape(B, NCHUNK * CHUNK // 2)
        y[:, k * (NOWN // 2):(k + 1) * (NOWN // 2)] = o
    return y, res


def kernel(x, toggle_gates):
    y, _ = _run(x, toggle_gates)
    return y
